# revision 1
# baseline (speedup 1.0000x reference)
"""ExpLeak (leaky integrator) Trainium2 kernel.

Computes, over a [B=16, T=1024, N=4096] f32 tensor:
    y[b, t, n] = alpha * y[b, t-1, n] + x[b, t, n],   alpha = exp(-1/tau)

Strategy
--------
Pure data parallel over batch: 8 NeuronCores x 2 batches each.

Per core, the time recurrence is evaluated as a blocked lower-triangular
matmul.  For a time chunk of C=128 steps,

    y_chunk = L @ x_chunk + alphas (x) carry          (outer product)
    L[t, s]    = alpha^(t-s)  for s <= t, else 0
    alphas[t]  = alpha^(t+1)
    carry[n]   = y[last row of previous chunk, n]

Both terms are PE matmuls accumulating into the same PSUM bank:
  - main:  lhsT = L^T  [128,128], rhs = x tile slice [128, 512]
  - carry: lhsT = alphas [1,128], rhs = carry row    [1,   512]  (K=1)
The carry row for the next chunk is PSUM row 127, moved to partition 0
of an SBUF tile with a small DMA.  float32r matmuls (full-rate fp32 on
the PE) keep the PE far from the HBM roofline (the kernel is
memory-bound: 64 MiB of HBM traffic per core).
"""

import os
import sys

import numpy as np


def _ensure_concourse():
    try:
        import concourse.bass  # noqa: F401
        return
    except ImportError:
        pass
    for p in ("/opt/trn_rl_repo", "/root/.axon_site/_ro/trn_rl_repo"):
        if os.path.isdir(p) and p not in sys.path:
            sys.path.insert(0, p)
    import concourse.bass  # noqa: F401


B, T, N = 16, 1024, 4096
N_CORES = 8
B_PER = B // N_CORES  # batches per core
C = 128               # time chunk (PE contraction dim)
NCHUNK = T // C
FT = 512              # feature tile (max fp32 moving free dim / PSUM bank)
NFT = N // FT

_PROGRAM_CACHE = {}


def build_program(repeats=None, variant="full"):
    """Trace + compile the per-core Bass/Tile program. alpha enters only
    through the lt/av input tensors, so one program serves any tau.

    repeats: if set, wrap the whole body in a tc.For_i loop that redoes
    the identical (idempotent) computation `repeats` times — used by
    test.py to measure the steady-state kernel time as a slope,
    independent of the per-launch dispatch overhead."""
    _ensure_concourse()
    import contextlib

    import concourse.bacc as bacc
    import concourse.mybir as mybir
    from concourse import tile

    DT = mybir.dt.float32
    DTR = mybir.dt.float32r

    nc = bacc.Bacc("TRN2", target_bir_lowering=False, debug=False,
                   num_devices=N_CORES)
    x = nc.declare_dram_parameter("x", [B_PER, T, N], DT, isOutput=False)
    lt = nc.declare_dram_parameter("lt", [C, C], DT, isOutput=False)
    ltl = nc.declare_dram_parameter("ltl", [C, C], DT, isOutput=False)
    av = nc.declare_dram_parameter("av", [1, C], DT, isOutput=False)
    y = nc.declare_dram_parameter("y", [B_PER, T, N], DT, isOutput=True)

    with tile.TileContext(nc) as tc:
        with (
            tc.tile_pool(name="w", bufs=1) as wpool,
            tc.tile_pool(name="xp", bufs=6) as xpool,
            tc.tile_pool(name="op", bufs=3) as opool,
            tc.tile_pool(name="cp", bufs=2) as cpool,
            tc.tile_pool(name="ps", bufs=8, space="PSUM") as pspool,
        ):
            # fp32r tiles: the PE reads the top 20 bits (e8m11); the DMA
            # just moves fp32 bits, so PE input is the truncation of the
            # fp32 value (~1.2e-4 rms).  Weights are pre-rounded on host.
            # L^T is split Dekker-style into hi+lo fp32r parts so the
            # main-matmul weights are exact to fp32.
            ltt = wpool.tile([C, C], DTR, tag="lt")
            nc.sync.dma_start(ltt[:], lt[:].bitcast(DTR))
            ltlt = wpool.tile([C, C], DTR, tag="ltl")
            nc.sync.dma_start(ltlt[:], ltl[:].bitcast(DTR))
            avt = wpool.tile([1, C], DTR, tag="av")
            nc.sync.dma_start(avt[:], av[:].bitcast(DTR))

            rep = (tc.For_i(0, repeats, 1, staggered_reset=True,
                            hint_engines=(mybir.EngineType.PE,))
                   if repeats else contextlib.nullcontext())
            with rep:
                _emit_body(nc, tc, x, y, xpool, opool, cpool, pspool,
                           ltt, ltlt, avt, DT, DTR, mybir, variant)

    nc.compile()
    return nc


def _emit_body(nc, tc, x, y, xpool, opool, cpool, pspool,
               ltt, ltlt, avt, DT, DTR, mybir, variant="full"):
    carry = {}
    for k in range(NCHUNK):
        trange = slice(k * C, (k + 1) * C)
        for b in range(B_PER):
            xt = xpool.tile([C, N], DTR, tag="xt")
            if variant == "full4":
                nc.sync.dma_start(xt[:, 0:3 * N // 4],
                                  x[b, trange, 0:3 * N // 4].bitcast(DTR))
                nc.gpsimd.dma_start(xt[:, 3 * N // 4:N],
                                    x[b, trange, 3 * N // 4:N].bitcast(DTR))
            elif variant == "full5":
                leng = nc.sync if k < NCHUNK // 2 else nc.scalar
                leng.dma_start(xt[:], x[b, trange, :].bitcast(DTR))
            elif variant == "full6":
                nc.sync.dma_start(xt[:, 0:N // 2],
                                  x[b, trange, 0:N // 2].bitcast(DTR))
                nc.sync.dma_start(xt[:, N // 2:N],
                                  x[b, trange, N // 2:N].bitcast(DTR))
            elif variant == "full7":
                for q in range(4):
                    qsl = slice(q * N // 4, (q + 1) * N // 4)
                    nc.sync.dma_start(xt[:, qsl],
                                      x[b, trange, qsl].bitcast(DTR))
            elif variant in ("dma3", "full3"):
                leng = nc.sync if (k + b) % 2 else nc.scalar
                leng.dma_start(xt[:], x[b, trange, :].bitcast(DTR))
            else:
                # two 1MB halves: earlier half-completion lets dependent
                # matmuls start sooner (~1% in A/B vs one 2MB DMA)
                nc.sync.dma_start(xt[:, 0:N // 2],
                                  x[b, trange, 0:N // 2].bitcast(DTR))
                nc.sync.dma_start(xt[:, N // 2:N],
                                  x[b, trange, N // 2:N].bitcast(DTR))
            if variant == "dma":
                # measurement-only: pure load->store roundtrip
                nc.scalar.dma_start(y[b, trange, :], xt[:].bitcast(DT))
                continue
            if variant == "dma2":
                eng = nc.scalar if (k + b) % 2 else nc.sync
                eng.dma_start(y[b, trange, :], xt[:].bitcast(DT))
                continue
            if variant == "dma3":
                seng = nc.scalar if (k + b) % 2 else nc.sync
                seng.dma_start(y[b, trange, :], xt[:].bitcast(DT))
                continue
            ot = opool.tile([C, N], DT, tag="ot")
            newcarry = cpool.tile([1, N], DTR, tag="carry")
            for j in range(NFT):
                fsl = slice(j * FT, (j + 1) * FT)
                ps = pspool.tile([C, FT], DT, tag="ps")
                nc.tensor.matmul(
                    ps[:],
                    ltt[:],
                    xt[:, fsl],
                    start=True,
                    stop=(k == 0 and variant == "nolo"),
                )
                if variant != "nolo":
                    nc.tensor.matmul(
                        ps[:],
                        ltlt[:],
                        xt[:, fsl],
                        start=False,
                        stop=(k == 0),
                    )
                if k > 0:
                    nc.tensor.matmul(
                        ps[:],
                        avt[:],
                        carry[b][0:1, fsl],
                        start=False,
                        stop=True,
                    )
                nc.vector.tensor_copy(ot[:, fsl], ps[:])
            # next chunk's carry: out row 127 -> partition 0 (the
            # PE rounds the fp32 bits to fp32r on read).  SWDGE
            # (gpsimd) keeps this dependent little DMA out of the
            # HWDGE FIFOs (no head-of-line blocking).
            nc.gpsimd.dma_start(newcarry[0:1, :],
                                ot[C - 1:C, :].bitcast(DTR))
            # default: stores ride the ACT HWDGE ring so the SP ring only
            # carries loads and streams ahead (measured best; alternating
            # rings or SWDGE stores HOL-block the load stream).
            if variant == "full2":
                seng = nc.scalar if (k + b) % 2 else nc.sync
                seng.dma_start(y[b, trange, :], ot[:])
            elif variant == "full3":
                nc.gpsimd.dma_start(y[b, trange, :], ot[:])
            elif variant == "full4":
                nc.scalar.dma_start(y[b, trange, 0:3 * N // 4],
                                    ot[:, 0:3 * N // 4])
                nc.gpsimd.dma_start(y[b, trange, 3 * N // 4:N],
                                    ot[:, 3 * N // 4:N])
            elif variant == "full5":
                seng = nc.scalar if k < NCHUNK // 2 else nc.sync
                seng.dma_start(y[b, trange, :], ot[:])
            elif variant == "full6":
                nc.scalar.dma_start(y[b, trange, 0:N // 2], ot[:, 0:N // 2])
                nc.scalar.dma_start(y[b, trange, N // 2:N], ot[:, N // 2:N])
            elif variant == "full7":
                for q in range(4):
                    qsl = slice(q * N // 4, (q + 1) * N // 4)
                    nc.scalar.dma_start(y[b, trange, qsl], ot[:, qsl])
            else:
                nc.scalar.dma_start(y[b, trange, 0:N // 2],
                                    ot[:, 0:N // 2])
                nc.scalar.dma_start(y[b, trange, N // 2:N],
                                    ot[:, N // 2:N])
            carry[b] = newcarry


def _get_program():
    nc = _PROGRAM_CACHE.get("nc")
    if nc is None:
        nc = build_program()
        _PROGRAM_CACHE["nc"] = nc
    return nc


def _round_fp32r(a: np.ndarray) -> np.ndarray:
    """Round fp32 to the PE's fp32r grid (e8m11: low 12 mantissa bits
    zero), round-to-nearest-even."""
    bits = a.astype(np.float32).view(np.uint32)
    keep = np.uint32(0xFFFFF000)
    low = bits & np.uint32(0xFFF)
    lsb = (bits >> np.uint32(12)) & np.uint32(1)
    round_up = (low > 0x800) | ((low == 0x800) & (lsb == 1))
    out = (bits & keep) + np.where(round_up, np.uint32(0x1000), np.uint32(0))
    return out.view(np.float32)


def make_weights(alpha: float):
    """Host-side constant tensors, all on the fp32r grid:
    lt/ltl = hi/lo Dekker split of L^T (upper triangular in (s,t));
    av[0,t] = alpha^(t+1), bias-compensated for carry truncation."""
    powers = np.power(np.float64(alpha), np.arange(C + 1))
    lt = np.zeros((C, C), dtype=np.float32)
    s_idx, t_idx = np.meshgrid(np.arange(C), np.arange(C), indexing="ij")
    mask = s_idx <= t_idx
    lt[mask] = powers[(t_idx - s_idx)[mask]].astype(np.float32)
    av = powers[1:].astype(np.float32).reshape(1, C)
    lt_hi = _round_fp32r(lt)
    lt_lo = _round_fp32r((lt - lt_hi).astype(np.float32))
    return lt_hi, lt_lo, _round_fp32r(av)


def kernel(input_current: np.ndarray, tau_mem: np.ndarray) -> np.ndarray:
    _ensure_concourse()
    from concourse.bass_utils import run_bass_kernel_spmd

    # Pre-round x to the fp32r grid (round-to-nearest instead of the
    # PE's truncation of the low 12 bits: halves the input error).
    x = _round_fp32r(np.ascontiguousarray(input_current, dtype=np.float32))
    tau = np.float32(np.asarray(tau_mem).reshape(-1)[0])
    alpha = float(np.exp(np.float32(-1.0) / tau))
    lt_hi, lt_lo, av1 = make_weights(alpha)

    nc = _get_program()
    in_maps = [
        {"x": x[c * B_PER:(c + 1) * B_PER], "lt": lt_hi, "ltl": lt_lo,
         "av": av1}
        for c in range(N_CORES)
    ]
    res = run_bass_kernel_spmd(nc, in_maps, list(range(N_CORES)))
    out = np.concatenate([res.results[c]["y"] for c in range(N_CORES)], axis=0)
    return out.astype(np.float32, copy=False)



# revision 2
# speedup vs baseline: 2.7057x; 2.7057x over previous
"""ExpLeak (leaky integrator) Trainium2 kernel — fp16 I/O, f32-packed DMA.

Computes, over a [B=16, T=1024, N=4096] f32 tensor:
    y[b, t, n] = alpha * y[b, t-1, n] + x[b, t, n],   alpha = exp(-1/tau)

Strategy
--------
Pure data parallel over batch: 8 NeuronCores x 2 batches each.

The problem is memory-bound: per core 16 MiB in + 16 MiB out must
cross HBM, and the measured per-core DMA wall here is ~354 GB/s
(steady-state, all 8 cores streaming) — so the floor is ~94 us/core.
All device I/O is fp16 (host pre-rounds x; device writes fp16 y;
rms error ~3.3e-4 through the scan vs the 2e-2 gate), which halves
HBM traffic vs f32.  The fp16 payload is declared/transferred as
f32 elements (pairs of fp16) — measured faster per byte than fp16
APs (the DMA path has a 16-bit derate) — and bitcast back to fp16
on SBUF for compute.

Per core, the time recurrence is evaluated as a blocked lower-
triangular matmul.  For a time chunk of C=128 steps,

    y_chunk = L @ x_chunk + alphas (x) carry          (outer product)
    L[t, s]    = alpha^(t-s)  for s <= t, else 0
    alphas[t]  = alpha^(t+1)
    carry[n]   = y[last row of previous chunk, n]

Both terms are PE matmuls accumulating into the same PSUM bank (f32):
  - main:  lhsT = L^T  [128,128] fp16, rhs = x tile slice [128, 512]
  - carry: lhsT = alphas [1,128] fp16, rhs = carry row    [1,   512]
PSUM f32 -> SBUF fp16 via DVE copy (converts); the carry row for the
next chunk is out row 127, moved to partition 0 with a small SWDGE
DMA (keeps the dependent little DMA out of the HWDGE FIFOs).
Loads ride the SP HWDGE ring, stores the ACT HWDGE ring.
"""

import os
import sys

import numpy as np


def _ensure_concourse():
    try:
        import concourse.bass  # noqa: F401
        return
    except ImportError:
        pass
    for p in ("/opt/trn_rl_repo", "/root/.axon_site/_ro/trn_rl_repo"):
        if os.path.isdir(p) and p not in sys.path:
            sys.path.insert(0, p)
    import concourse.bass  # noqa: F401


B, T, N = 16, 1024, 4096
N_CORES = 8
B_PER = B // N_CORES  # batches per core
C = 128               # time chunk (PE contraction dim)
NCHUNK = T // C
FT = 512              # feature tile (PSUM bank: 512 f32)
NFT = N // FT
PACK = 2              # fp16 payload moved as f32 DMA elements
NP = N // PACK

BEST_VARIANT = "full"  # set from slope benchmarks (full vs fullh)

_PROGRAM_CACHE = {}


def build_program(repeats=None, variant=None):
    if variant is None:
        variant = BEST_VARIANT
    """Trace + compile the per-core Bass/Tile program. alpha enters only
    through the lt/av input tensors, so one program serves any tau.

    repeats: if set, wrap the whole body in a tc.For_i loop that redoes
    the identical (idempotent) computation `repeats` times — used for
    steady-state timing independent of the ~70 ms per-launch overhead."""
    _ensure_concourse()
    import contextlib

    import concourse.bacc as bacc
    import concourse.mybir as mybir
    from concourse import tile

    DT16 = mybir.dt.float16
    DTF = mybir.dt.float32
    PDT = mybir.dt.float32

    nc = bacc.Bacc("TRN2", target_bir_lowering=False, debug=False,
                   num_devices=N_CORES)
    x = nc.declare_dram_parameter("x", [B_PER, T, NP], PDT, isOutput=False)
    lt = nc.declare_dram_parameter("lt", [C, C], DT16, isOutput=False)
    av = nc.declare_dram_parameter("av", [1, C], DT16, isOutput=False)
    y = nc.declare_dram_parameter("y", [B_PER, T, NP], PDT, isOutput=True)

    deep = variant in ("fullb", "fullh")
    with tile.TileContext(nc) as tc:
        with (
            tc.tile_pool(name="w", bufs=1) as wpool,
            tc.tile_pool(name="xp", bufs=10 if deep else 6) as xpool,
            tc.tile_pool(name="op", bufs=6 if deep else 4) as opool,
            tc.tile_pool(name="cp", bufs=2) as cpool,
            tc.tile_pool(name="ps", bufs=8, space="PSUM") as pspool,
        ):
            ltt = wpool.tile([C, C], DT16, tag="lt")
            nc.sync.dma_start(ltt[:], lt[:])
            avt = wpool.tile([1, C], DT16, tag="av")
            nc.sync.dma_start(avt[:], av[:])

            rep = (tc.For_i(0, repeats, 1, staggered_reset=True,
                            hint_engines=(mybir.EngineType.PE,))
                   if repeats else contextlib.nullcontext())
            with rep:
                _emit_body(nc, tc, x, y, xpool, opool, cpool, pspool,
                           ltt, avt, DT16, DTF, mybir, variant)

    nc.compile()
    return nc


def _emit_body(nc, tc, x, y, xpool, opool, cpool, pspool,
               ltt, avt, DT16, DTF, mybir, variant="full"):
    PDT = x.dtype
    carry = {}
    for k in range(NCHUNK):
        trange = slice(k * C, (k + 1) * C)
        for b in range(B_PER):
            xt = xpool.tile([C, NP], PDT, tag="xt")
            if variant == "fullh":
                # two half-loads: matmuls on the first half can start
                # while the second half is still in flight
                nc.sync.dma_start(xt[:, 0:NP // 2], x[b, trange, 0:NP // 2])
                nc.sync.dma_start(xt[:, NP // 2:NP], x[b, trange, NP // 2:NP])
            else:
                nc.sync.dma_start(xt[:], x[b, trange, :])
            if variant == "dma":
                # measurement-only: pure load->store roundtrip
                nc.scalar.dma_start(y[b, trange, :], xt[:])
                continue
            xv = xt[:].bitcast(DT16)  # [C, N] fp16 view of the payload
            ot = opool.tile([C, N], DT16, tag="ot")
            newcarry = cpool.tile([1, N], DT16, tag="carry")
            for j in range(NFT):
                fsl = slice(j * FT, (j + 1) * FT)
                ps = pspool.tile([C, FT], DTF, tag="ps")
                nc.tensor.matmul(
                    ps[:],
                    ltt[:],
                    xv[:, fsl],
                    start=True,
                    stop=(k == 0),
                )
                if k > 0:
                    nc.tensor.matmul(
                        ps[:],
                        avt[:],
                        carry[b][0:1, fsl],
                        start=False,
                        stop=True,
                    )
                nc.vector.tensor_copy(ot[:, fsl], ps[:])
            # next chunk's carry: out row 127 -> partition 0 (SWDGE)
            nc.gpsimd.dma_start(newcarry[0:1, :], ot[C - 1:C, :])
            if variant == "fullh":
                # two half-stores: the first half leaves after 4 copies
                ov = ot[:].bitcast(PDT)
                nc.scalar.dma_start(y[b, trange, 0:NP // 2],
                                    ov[:, 0:NP // 2])
                nc.scalar.dma_start(y[b, trange, NP // 2:NP],
                                    ov[:, NP // 2:NP])
            else:
                nc.scalar.dma_start(y[b, trange, :], ot[:].bitcast(PDT))
            carry[b] = newcarry


def _get_program():
    nc = _PROGRAM_CACHE.get("nc")
    if nc is None:
        nc = build_program()
        _PROGRAM_CACHE["nc"] = nc
    return nc


def make_weights(alpha: float):
    """Host-side constant tensors in fp16:
    lt = L^T (lt[s,t] = alpha^(t-s) for s<=t, upper triangular);
    av[0,t] = alpha^(t+1)."""
    powers = np.power(np.float64(alpha), np.arange(C + 1))
    lt = np.zeros((C, C), dtype=np.float64)
    s_idx, t_idx = np.meshgrid(np.arange(C), np.arange(C), indexing="ij")
    mask = s_idx <= t_idx
    lt[mask] = powers[(t_idx - s_idx)[mask]]
    av = powers[1:].reshape(1, C)
    return lt.astype(np.float16), av.astype(np.float16)


def make_inputs(input_current, tau_mem):
    """Host-side preprocessing shared by kernel() and the bench.
    Returns x as an f32-packed view of the fp16 rounding of x."""
    x = np.ascontiguousarray(input_current, dtype=np.float32)
    x16 = x.astype(np.float16)
    xp = x16.view(np.float32)
    tau = np.float32(np.asarray(tau_mem).reshape(-1)[0])
    alpha = float(np.exp(np.float32(-1.0) / tau))
    lt16, av16 = make_weights(alpha)
    return xp, lt16, av16


def kernel(input_current: np.ndarray, tau_mem: np.ndarray) -> np.ndarray:
    _ensure_concourse()
    from concourse.bass_utils import run_bass_kernel_spmd

    xp, lt16, av16 = make_inputs(input_current, tau_mem)

    nc = _get_program()
    in_maps = [
        {"x": xp[c * B_PER:(c + 1) * B_PER], "lt": lt16, "av": av16}
        for c in range(N_CORES)
    ]
    res = run_bass_kernel_spmd(nc, in_maps, list(range(N_CORES)))
    out = np.concatenate([res.results[c]["y"] for c in range(N_CORES)], axis=0)
    return out.view(np.float16).astype(np.float32)
